# revision 17
# baseline (speedup 1.0000x reference)
"""Bayesian linear layer (reparameterized sample + KL) on 8 Trainium2 NeuronCores.

Reference computation (all fp32):
    weight = weight_mu + exp(weight_sigma) * eps_w          # [OUT, IN]
    bias   = bias_mu   + exp(bias_sigma)   * eps_b          # [OUT]
    out    = x @ weight.T + bias                            # [B, OUT]
    kl     = -0.5 * sum(1 + 2*ws - wm^2 - exp(2*ws))        # over weight
             -0.5 * sum(1 + 2*bs - bm^2 - exp(2*bs))        # over bias

Sharding: 2x4 mesh.  Batch split in 2 (B_S=4096 rows/core), out_features in 4
(O_S=1024 cols/core); core c handles b_shard=c//4, o_shard=c%4.  This gives the
lowest per-core HBM traffic (x shard 67MB + weight params 50MB + out 17MB) and
keeps the kernel PE-bound on float32r matmuls (1 col/cycle at N=512).

Host-side layout work (part of the sharding strategy):
  - x is passed pre-transposed AND pre-tiled as [BT, 128, KT, 128] so each
    batch-tile load is one DMA with 16KB-contiguous per-partition runs.
  - weight mu/sigma/eps are interleaved per k-tile as [KT, 128, 3, O_S]
    (12KB-contiguous per partition) -> one DMA per k-tile.
The contraction dim lands on SBUF partitions with no on-device transposes.

Device kernel per core:
  phase 1: per k-tile, load packed params (scalar-engine HWDGE ring), build
           W^T tile (W = mu + exp(sigma)*eps, rounded once to f32r) and
           accumulate KL partial sums on ACT/DVE.
  phase 2: per 128-row batch tile, load x tile (sync HWDGE ring), accumulate
           out[128, O_S] over 32 k-tiles into PSUM, add bias, store (SWDGE).
  tail:    fold KL partials into one scalar.
"""

import sys

import numpy as np

try:
    import concourse.bass as bass  # noqa: F401
except ImportError:  # pragma: no cover
    sys.path.insert(0, "/opt/trn_rl_repo")

import concourse.bass as bass
import concourse.tile as tile
from concourse import bacc, bass_isa, mybir

P = 128
B_FULL, IN_FULL, OUT_FULL = 8192, 4096, 4096
B_SHARDS, O_SHARDS = 2, 4
N_CORES = 8

F32 = mybir.dt.float32
MM_DT = mybir.dt.float32r  # PE fast-fp32 mode: 1 cycle/row at N>=256

AF = mybir.ActivationFunctionType
OP = mybir.AluOpType
AX = mybir.AxisListType


def build_bayes_kernel(nc, IN, B_S, O_S, mm_dt=MM_DT):
    """Emit the per-core SPMD program. Tensors are declared on nc."""
    KT = IN // P        # k tiles
    BT = B_S // P       # batch tiles
    NFREE = 512         # matmul moving free dim (1 psum bank of fp32)
    OC = O_S // NFREE   # out chunks per batch tile

    xt = nc.dram_tensor("xt", [BT, P, KT, P], mm_dt, kind="ExternalInput").ap()
    wpk = nc.dram_tensor("wpk", [KT, P, 3, O_S], F32, kind="ExternalInput").ap()
    bpk = nc.dram_tensor("bpk", [1, 3, O_S], F32, kind="ExternalInput").ap()
    out = nc.dram_tensor("out", [B_S, O_S], F32, kind="ExternalOutput").ap()
    kl = nc.dram_tensor("kl", [1, 1], F32, kind="ExternalOutput").ap()

    out_r = out.rearrange("(bt p) o -> p bt o", p=P)

    with tile.TileContext(nc) as tc:
        with (
            tc.tile_pool(name="wpool", bufs=1) as wpool,
            tc.tile_pool(name="gen", bufs=2) as gen,
            tc.tile_pool(name="xin", bufs=2) as xin,
            tc.tile_pool(name="ost", bufs=2) as ost,
            tc.tile_pool(name="misc", bufs=1) as misc,
            tc.tile_pool(name="psum", bufs=4, space="PSUM") as psum,
            tc.tile_pool(name="pgps", bufs=1, space="PSUM") as pgps,
        ):
            # Persistent state
            w_tiles = [
                wpool.tile([P, O_S], mm_dt, tag=f"w{kt}", name=f"w{kt}")
                for kt in range(KT)
            ]
            ssig = misc.tile([P, KT], F32, tag="ssig")   # per-ktile sum(sigma)
            smu2 = misc.tile([P, KT], F32, tag="smu2")   # sum(mu^2)
            sv2 = misc.tile([P, KT], F32, tag="sv2")     # sum(exp(2 sigma))
            b_bc = misc.tile([P, O_S], F32, tag="bbc")   # bias broadcast
            klb = misc.tile([1, 4], F32, tag="klb")      # bias kl: ssig, smu2, sv2, tmp

            # Prefetch the first x tiles on the sync ring; they flow while the
            # scalar ring streams the 50MB of weight params, and they feed the
            # PE prologue below.
            PG = min(2, BT)
            xpre = []
            for bt in range(PG):
                xs = xin.tile([P, KT, P], mm_dt, tag="xs", name=f"xpre{bt}")
                nc.sync.dma_start(xs, xt[bt])
                xpre.append(xs)
            pg_ps = [
                [
                    pgps.tile([P, NFREE], F32, tag=f"pg{pb}_{oc}", name=f"pg{pb}_{oc}")
                    for oc in range(OC)
                ]
                for pb in range(PG)
            ]

            # ---- Phase 1: W = mu + exp(sigma)*eps, KL partial sums ----
            # One ACT->DVE hop per k-tile, then all DVE work back-to-back.
            # The first PG batch tiles accumulate their matmuls k-tile by
            # k-tile right here (PE prologue): the PE consumes each W tile as
            # it is produced instead of idling until phase 1 completes.
            for kt in range(KT):
                g = gen.tile([P, 3, O_S], F32, tag="wpk")
                nc.scalar.dma_start(g, wpk[kt])
                sig, mu, eps = g[:, 0, :], g[:, 1, :], g[:, 2, :]
                wtmp = gen.tile([P, O_S], F32, tag="wtmp")
                nc.scalar.activation(wtmp, sig, AF.Exp)             # v
                nc.vector.tensor_reduce(ssig[:, kt : kt + 1], sig, AX.X, OP.add)
                # v^2 summed; tensor output dumps over the dead sigma slot
                nc.vector.scalar_tensor_tensor(
                    sig, wtmp, 1.0, wtmp, OP.mult, OP.mult,
                    accum_out=sv2[:, kt : kt + 1],
                )
                nc.vector.tensor_tensor(wtmp, wtmp, eps, OP.mult)
                # final add writes the f32r matmul operand (single rounding)
                nc.vector.tensor_tensor(w_tiles[kt], wtmp, mu, OP.add)
                # mu^2 summed; dump lands in the dead eps slot
                nc.scalar.activation(
                    eps, mu, AF.Square, accum_out=smu2[:, kt : kt + 1]
                )
                for pb in range(PG):
                    for oc in range(OC):
                        sl = slice(oc * NFREE, (oc + 1) * NFREE)
                        nc.tensor.matmul(
                            pg_ps[pb][oc],
                            xpre[pb][:, kt, :],
                            w_tiles[kt][:, sl],
                            start=(kt == 0),
                            stop=(kt == KT - 1),
                        )

            # ---- Bias: value + KL pieces ----
            # all SBUF operands of an op share start partition 0; dead slots
            # of the packed tile double as ACT dump targets.
            bt_ = gen.tile([1, 3, O_S], F32, tag="wpk", name="biastile")
            nc.sync.dma_start(bt_, bpk[0])
            bsig, bmu, beps = bt_[:, 0, :], bt_[:, 1, :], bt_[:, 2, :]
            bv = b_bc[0:1, :]
            nc.vector.tensor_reduce(klb[:, 0:1], bsig, AX.X, OP.add)
            nc.scalar.activation(bv, bsig, AF.Exp)
            nc.vector.tensor_tensor(bv, bv, beps, OP.mult)   # beps dead after
            nc.vector.tensor_tensor(bv, bv, bmu, OP.add)
            nc.scalar.activation(
                beps, bsig, AF.Exp, scale=2.0, accum_out=klb[:, 2:3]
            )
            nc.scalar.activation(bsig, bmu, AF.Square, accum_out=klb[:, 1:2])
            nc.gpsimd.partition_broadcast(b_bc, bv)

            # ---- Prologue eviction: bias-add + store for the PG tiles ----
            for pb in range(PG):
                osb = ost.tile([P, O_S], F32, tag="osb", name=f"osb_pg{pb}")
                for oc in range(OC):
                    sl = slice(oc * NFREE, (oc + 1) * NFREE)
                    nc.vector.tensor_tensor(
                        osb[:, sl], pg_ps[pb][oc], b_bc[:, sl], OP.add
                    )
                nc.sync.dma_start(out_r[:, pb, :], osb)

            # ---- Phase 2: out[bt] = x[bt] @ W^T + bias ----
            for bt in range(PG, BT):
                xs = xin.tile([P, KT, P], mm_dt, tag="xs")
                nc.sync.dma_start(xs, xt[bt])
                osb = ost.tile([P, O_S], F32, tag="osb")
                for oc in range(OC):
                    sl = slice(oc * NFREE, (oc + 1) * NFREE)
                    ps = psum.tile([P, NFREE], F32, tag="ps")
                    for kt in range(KT):
                        nc.tensor.matmul(
                            ps,
                            xs[:, kt, :],
                            w_tiles[kt][:, sl],
                            start=(kt == 0),
                            stop=(kt == KT - 1),
                        )
                    nc.vector.tensor_tensor(osb[:, sl], ps, b_bc[:, sl], OP.add)
                nc.sync.dma_start(out_r[:, bt, :], osb)

            # ---- KL tail ----
            rs = misc.tile([P, 1], F32, tag="rs")
            rm = misc.tile([P, 1], F32, tag="rm")
            rv = misc.tile([P, 1], F32, tag="rv")
            nc.vector.tensor_reduce(rs, ssig, AX.X, OP.add)
            nc.vector.tensor_reduce(rm, smu2, AX.X, OP.add)
            nc.vector.tensor_reduce(rv, sv2, AX.X, OP.add)
            tcol = misc.tile([P, 1], F32, tag="tcol")
            # tcol = 2*rs - rm - rv
            nc.vector.scalar_tensor_tensor(tcol, rs, 2.0, rm, OP.mult, OP.subtract)
            nc.vector.tensor_tensor(tcol, tcol, rv, OP.subtract)
            # bias terms fold into partition 0
            nc.vector.scalar_tensor_tensor(
                klb[:, 3:4], klb[:, 0:1], 2.0, klb[:, 1:2], OP.mult, OP.subtract
            )
            nc.vector.tensor_tensor(klb[:, 3:4], klb[:, 3:4], klb[:, 2:3], OP.subtract)
            nc.vector.tensor_tensor(tcol[0:1, :], tcol[0:1, :], klb[:, 3:4], OP.add)
            tall = misc.tile([P, 1], F32, tag="tall")
            nc.gpsimd.partition_all_reduce(tall, tcol, P, bass_isa.ReduceOp.add)
            # kl = -0.5 * (count + sum(2s - m^2 - v^2))
            count = float(IN * O_S + O_S)
            klt = misc.tile([1, 1], F32, tag="klt")
            nc.vector.tensor_scalar(klt, tall[0:1, :], count, -0.5, OP.add, OP.mult)
            nc.sync.dma_start(kl, klt)


_NC_CACHE = {}


def _get_nc():
    key = "full"
    if key not in _NC_CACHE:
        nc = bacc.Bacc("TRN2", target_bir_lowering=False, debug=False)
        build_bayes_kernel(nc, IN_FULL, B_FULL // B_SHARDS, OUT_FULL // O_SHARDS)
        nc.compile()
        _NC_CACHE[key] = nc
    return _NC_CACHE[key]


def _pack_x(x_shard, BT, KT):
    # [B_S, IN] -> [BT, P, KT, P] with [bt, p(=feature in tile), kt, b]
    x4 = x_shard.reshape(BT, P, KT, P)          # [bt, b, kt, p_feature]
    return np.ascontiguousarray(x4.transpose(0, 3, 2, 1))


def _pack_w(sig, mu, eps, KT, O_S):
    # each [O_S, IN] -> packed [KT, P, 3, O_S] with feature on partition
    stk = np.stack([sig.T, mu.T, eps.T], axis=1)   # [IN, 3, O_S]
    return np.ascontiguousarray(stk.reshape(KT, P, 3, O_S))


def _shard_inputs(x, weight_mu, weight_sigma, bias_mu, bias_sigma, eps_w, eps_b):
    B_S = B_FULL // B_SHARDS
    O_S = OUT_FULL // O_SHARDS
    BT, KT = B_S // P, IN_FULL // P
    f = np.float32
    x = np.asarray(x, dtype=f)
    weight_mu = np.asarray(weight_mu, dtype=f)
    weight_sigma = np.asarray(weight_sigma, dtype=f)
    eps_w = np.asarray(eps_w, dtype=f)
    bias_mu = np.asarray(bias_mu, dtype=f)
    bias_sigma = np.asarray(bias_sigma, dtype=f)
    eps_b = np.asarray(eps_b, dtype=f)

    xb = [_pack_x(x[b * B_S : (b + 1) * B_S], BT, KT) for b in range(B_SHARDS)]
    wb, bb = [], []
    for o in range(O_SHARDS):
        osl = slice(o * O_S, (o + 1) * O_S)
        wb.append(
            _pack_w(weight_sigma[osl], weight_mu[osl], eps_w[osl], KT, O_S)
        )
        bb.append(
            np.ascontiguousarray(
                np.stack([bias_sigma[osl], bias_mu[osl], eps_b[osl]])[None]
            )
        )

    in_maps = []
    for c in range(N_CORES):
        b, o = divmod(c, O_SHARDS)
        in_maps.append({"xt": xb[b], "wpk": wb[o], "bpk": bb[o]})
    return in_maps


def kernel(x, weight_mu, weight_sigma, bias_mu, bias_sigma, eps_w, eps_b, **run_kwargs):
    from concourse.bass_utils import run_bass_kernel_spmd

    B_S = B_FULL // B_SHARDS
    O_S = OUT_FULL // O_SHARDS
    in_maps = _shard_inputs(
        x, weight_mu, weight_sigma, bias_mu, bias_sigma, eps_w, eps_b
    )
    nc = _get_nc()
    res = run_bass_kernel_spmd(nc, in_maps, core_ids=list(range(N_CORES)), **run_kwargs)
    out = np.empty((B_FULL, OUT_FULL), np.float32)
    for c in range(N_CORES):
        b, o = divmod(c, O_SHARDS)
        out[b * B_S : (b + 1) * B_S, o * O_S : (o + 1) * O_S] = res.results[c]["out"]
    # each (weight, bias) o-shard's KL is identical on both mesh rows; take row 0
    kl_val = np.float32(sum(float(res.results[c]["kl"][0, 0]) for c in range(O_SHARDS)))
    if run_kwargs:
        kernel.last_results = res
    return out, kl_val


# revision 20
# speedup vs baseline: 1.0392x; 1.0392x over previous
"""Bayesian linear layer (reparameterized sample + KL) on 8 Trainium2 NeuronCores.

Reference computation (all fp32):
    weight = weight_mu + exp(weight_sigma) * eps_w          # [OUT, IN]
    bias   = bias_mu   + exp(bias_sigma)   * eps_b          # [OUT]
    out    = x @ weight.T + bias                            # [B, OUT]
    kl     = -0.5 * sum(1 + 2*ws - wm^2 - exp(2*ws))        # over weight
             -0.5 * sum(1 + 2*bs - bm^2 - exp(2*bs))        # over bias

Sharding: 2x4 mesh.  Batch split in 2 (B_S=4096 rows/core), out_features in 4
(O_S=1024 cols/core); core c handles b_shard=c//4, o_shard=c%4.  This gives the
lowest per-core HBM traffic (x shard 67MB + weight params 50MB + out 17MB) and
keeps the kernel PE-bound on float32r matmuls (1 col/cycle at N=512).

Host-side layout work (part of the sharding strategy):
  - x is passed pre-transposed AND pre-tiled as [BT, 128, KT, 128] so each
    batch-tile load is one DMA with 16KB-contiguous per-partition runs.
  - weight mu/sigma/eps are interleaved per k-tile as [KT, 128, 3, O_S]
    (12KB-contiguous per partition) -> one DMA per k-tile.
The contraction dim lands on SBUF partitions with no on-device transposes.

Device kernel per core:
  phase 1: per k-tile, load packed params (scalar-engine HWDGE ring), build
           W^T tile (W = mu + exp(sigma)*eps, rounded once to f32r) and
           accumulate KL partial sums on ACT/DVE.
  phase 2: per 128-row batch tile, load x tile (sync HWDGE ring), accumulate
           out[128, O_S] over 32 k-tiles into PSUM, add bias, store (SWDGE).
  tail:    fold KL partials into one scalar.
"""

import sys

import numpy as np

try:
    import concourse.bass as bass  # noqa: F401
except ImportError:  # pragma: no cover
    sys.path.insert(0, "/opt/trn_rl_repo")

import concourse.bass as bass
import concourse.tile as tile
from concourse import bacc, bass_isa, mybir

P = 128
B_FULL, IN_FULL, OUT_FULL = 8192, 4096, 4096
B_SHARDS, O_SHARDS = 2, 4
N_CORES = 8

F32 = mybir.dt.float32
MM_DT = mybir.dt.float32r  # PE fast-fp32 mode: 1 cycle/row at N>=256

AF = mybir.ActivationFunctionType
OP = mybir.AluOpType
AX = mybir.AxisListType


def build_bayes_kernel(nc, IN, B_S, O_S, mm_dt=MM_DT):
    """Emit the per-core SPMD program. Tensors are declared on nc."""
    KT = IN // P        # k tiles
    BT = B_S // P       # batch tiles
    NFREE = 512         # matmul moving free dim (1 psum bank of fp32)
    OC = O_S // NFREE   # out chunks per batch tile

    xt = nc.dram_tensor("xt", [BT, P, KT, P], mm_dt, kind="ExternalInput").ap()
    wpk = nc.dram_tensor("wpk", [KT, P, 3, O_S], F32, kind="ExternalInput").ap()
    bpk = nc.dram_tensor("bpk", [1, 3, O_S], F32, kind="ExternalInput").ap()
    out = nc.dram_tensor("out", [B_S, O_S], F32, kind="ExternalOutput").ap()
    kl = nc.dram_tensor("kl", [1, 1], F32, kind="ExternalOutput").ap()

    out_r = out.rearrange("(bt p) o -> p bt o", p=P)

    with tile.TileContext(nc) as tc:
        with (
            tc.tile_pool(name="wpool", bufs=1) as wpool,
            tc.tile_pool(name="gen", bufs=2) as gen,
            tc.tile_pool(name="xin", bufs=2) as xin,
            tc.tile_pool(name="ost", bufs=2) as ost,
            tc.tile_pool(name="misc", bufs=1) as misc,
            tc.tile_pool(name="psum", bufs=4, space="PSUM") as psum,
            tc.tile_pool(name="pgps", bufs=1, space="PSUM") as pgps,
        ):
            # Persistent state
            w_tiles = [
                wpool.tile([P, O_S], mm_dt, tag=f"w{kt}", name=f"w{kt}")
                for kt in range(KT)
            ]
            ssig = misc.tile([P, KT], F32, tag="ssig")   # per-ktile sum(sigma)
            smu2 = misc.tile([P, KT], F32, tag="smu2")   # sum(mu^2)
            sv2 = misc.tile([P, KT], F32, tag="sv2")     # sum(exp(2 sigma))
            b_bc = misc.tile([P, O_S], F32, tag="bbc")   # bias broadcast
            klb = misc.tile([1, 4], F32, tag="klb")      # bias kl: ssig, smu2, sv2, tmp

            # Prefetch the first x tiles on the sync ring; they flow while the
            # scalar ring streams the 50MB of weight params, and they feed the
            # PE prologue below.
            PG = min(2, BT)
            xpre = []
            for bt in range(PG):
                xs = xin.tile([P, KT, P], mm_dt, tag="xs", name=f"xpre{bt}")
                nc.sync.dma_start(xs, xt[bt])
                xpre.append(xs)
            pg_ps = [
                [
                    pgps.tile([P, NFREE], F32, tag=f"pg{pb}_{oc}", name=f"pg{pb}_{oc}")
                    for oc in range(OC)
                ]
                for pb in range(PG)
            ]

            # ---- Phase 1: W = mu + exp(sigma)*eps, KL partial sums ----
            # One ACT->DVE hop per k-tile, then all DVE work back-to-back.
            # The first PG batch tiles accumulate their matmuls k-tile by
            # k-tile right here (PE prologue): the PE consumes each W tile as
            # it is produced instead of idling until phase 1 completes.
            for kt in range(KT):
                g = gen.tile([P, 3, O_S], F32, tag="wpk")
                nc.scalar.dma_start(g, wpk[kt])
                sig, mu, eps = g[:, 0, :], g[:, 1, :], g[:, 2, :]
                wtmp = gen.tile([P, O_S], F32, tag="wtmp")
                # ACT ops depend only on the DMA (dumps go to dedicated
                # scratch), so ACT's in-order queue never waits on DVE and
                # k-tiles pipeline cleanly.
                d1 = misc.tile([P, O_S], F32, tag="sqd")
                nc.scalar.activation(
                    d1, sig, AF.Identity, accum_out=ssig[:, kt : kt + 1]
                )
                nc.scalar.activation(wtmp, sig, AF.Exp)             # v
                d2 = misc.tile([P, O_S], F32, tag="sqd")
                nc.scalar.activation(
                    d2, mu, AF.Square, accum_out=smu2[:, kt : kt + 1]
                )
                # v^2 summed; tensor output dumps over the dead sigma slot
                nc.vector.scalar_tensor_tensor(
                    sig, wtmp, 1.0, wtmp, OP.mult, OP.mult,
                    accum_out=sv2[:, kt : kt + 1],
                )
                nc.vector.tensor_tensor(wtmp, wtmp, eps, OP.mult)
                # final add writes the f32r matmul operand (single rounding)
                nc.vector.tensor_tensor(w_tiles[kt], wtmp, mu, OP.add)
                for pb in range(PG):
                    for oc in range(OC):
                        sl = slice(oc * NFREE, (oc + 1) * NFREE)
                        nc.tensor.matmul(
                            pg_ps[pb][oc],
                            xpre[pb][:, kt, :],
                            w_tiles[kt][:, sl],
                            start=(kt == 0),
                            stop=(kt == KT - 1),
                        )

            # ---- Bias: value + KL pieces ----
            # all SBUF operands of an op share start partition 0; dead slots
            # of the packed tile double as ACT dump targets.
            bt_ = gen.tile([1, 3, O_S], F32, tag="wpk", name="biastile")
            nc.sync.dma_start(bt_, bpk[0])
            bsig, bmu, beps = bt_[:, 0, :], bt_[:, 1, :], bt_[:, 2, :]
            bv = b_bc[0:1, :]
            nc.vector.tensor_reduce(klb[:, 0:1], bsig, AX.X, OP.add)
            nc.scalar.activation(bv, bsig, AF.Exp)
            nc.vector.tensor_tensor(bv, bv, beps, OP.mult)   # beps dead after
            nc.vector.tensor_tensor(bv, bv, bmu, OP.add)
            nc.scalar.activation(
                beps, bsig, AF.Exp, scale=2.0, accum_out=klb[:, 2:3]
            )
            nc.scalar.activation(bsig, bmu, AF.Square, accum_out=klb[:, 1:2])
            nc.gpsimd.partition_broadcast(b_bc, bv)

            # ---- Prologue eviction: bias-add + store for the PG tiles ----
            for pb in range(PG):
                for oc in range(OC):
                    sl = slice(oc * NFREE, (oc + 1) * NFREE)
                    osb = ost.tile([P, NFREE], F32, tag="osb", name=f"osb_pg{pb}_{oc}")
                    nc.vector.tensor_tensor(osb, pg_ps[pb][oc], b_bc[:, sl], OP.add)
                    nc.sync.dma_start(out_r[:, pb, sl], osb)

            # ---- Phase 2: out[bt] = x[bt] @ W^T + bias ----
            for bt in range(PG, BT):
                xs = xin.tile([P, KT, P], mm_dt, tag="xs")
                nc.sync.dma_start(xs, xt[bt])
                for oc in range(OC):
                    sl = slice(oc * NFREE, (oc + 1) * NFREE)
                    ps = psum.tile([P, NFREE], F32, tag="ps")
                    for kt in range(KT):
                        nc.tensor.matmul(
                            ps,
                            xs[:, kt, :],
                            w_tiles[kt][:, sl],
                            start=(kt == 0),
                            stop=(kt == KT - 1),
                        )
                    osb = ost.tile([P, NFREE], F32, tag="osb")
                    nc.vector.tensor_tensor(osb, ps, b_bc[:, sl], OP.add)
                    nc.sync.dma_start(out_r[:, bt, sl], osb)

            # ---- KL tail ----
            rs = misc.tile([P, 1], F32, tag="rs")
            rm = misc.tile([P, 1], F32, tag="rm")
            rv = misc.tile([P, 1], F32, tag="rv")
            nc.vector.tensor_reduce(rs, ssig, AX.X, OP.add)
            nc.vector.tensor_reduce(rm, smu2, AX.X, OP.add)
            nc.vector.tensor_reduce(rv, sv2, AX.X, OP.add)
            tcol = misc.tile([P, 1], F32, tag="tcol")
            # tcol = 2*rs - rm - rv
            nc.vector.scalar_tensor_tensor(tcol, rs, 2.0, rm, OP.mult, OP.subtract)
            nc.vector.tensor_tensor(tcol, tcol, rv, OP.subtract)
            # bias terms fold into partition 0
            nc.vector.scalar_tensor_tensor(
                klb[:, 3:4], klb[:, 0:1], 2.0, klb[:, 1:2], OP.mult, OP.subtract
            )
            nc.vector.tensor_tensor(klb[:, 3:4], klb[:, 3:4], klb[:, 2:3], OP.subtract)
            nc.vector.tensor_tensor(tcol[0:1, :], tcol[0:1, :], klb[:, 3:4], OP.add)
            tall = misc.tile([P, 1], F32, tag="tall")
            nc.gpsimd.partition_all_reduce(tall, tcol, P, bass_isa.ReduceOp.add)
            # kl = -0.5 * (count + sum(2s - m^2 - v^2))
            count = float(IN * O_S + O_S)
            klt = misc.tile([1, 1], F32, tag="klt")
            nc.vector.tensor_scalar(klt, tall[0:1, :], count, -0.5, OP.add, OP.mult)
            nc.sync.dma_start(kl, klt)


_NC_CACHE = {}


def _get_nc():
    key = "full"
    if key not in _NC_CACHE:
        nc = bacc.Bacc("TRN2", target_bir_lowering=False, debug=False)
        build_bayes_kernel(nc, IN_FULL, B_FULL // B_SHARDS, OUT_FULL // O_SHARDS)
        nc.compile()
        _NC_CACHE[key] = nc
    return _NC_CACHE[key]


def _pack_x(x_shard, BT, KT):
    # [B_S, IN] -> [BT, P, KT, P] with [bt, p(=feature in tile), kt, b]
    x4 = x_shard.reshape(BT, P, KT, P)          # [bt, b, kt, p_feature]
    return np.ascontiguousarray(x4.transpose(0, 3, 2, 1))


def _pack_w(sig, mu, eps, KT, O_S):
    # each [O_S, IN] -> packed [KT, P, 3, O_S] with feature on partition
    stk = np.stack([sig.T, mu.T, eps.T], axis=1)   # [IN, 3, O_S]
    return np.ascontiguousarray(stk.reshape(KT, P, 3, O_S))


def _shard_inputs(x, weight_mu, weight_sigma, bias_mu, bias_sigma, eps_w, eps_b):
    B_S = B_FULL // B_SHARDS
    O_S = OUT_FULL // O_SHARDS
    BT, KT = B_S // P, IN_FULL // P
    f = np.float32
    x = np.asarray(x, dtype=f)
    weight_mu = np.asarray(weight_mu, dtype=f)
    weight_sigma = np.asarray(weight_sigma, dtype=f)
    eps_w = np.asarray(eps_w, dtype=f)
    bias_mu = np.asarray(bias_mu, dtype=f)
    bias_sigma = np.asarray(bias_sigma, dtype=f)
    eps_b = np.asarray(eps_b, dtype=f)

    xb = [_pack_x(x[b * B_S : (b + 1) * B_S], BT, KT) for b in range(B_SHARDS)]
    wb, bb = [], []
    for o in range(O_SHARDS):
        osl = slice(o * O_S, (o + 1) * O_S)
        wb.append(
            _pack_w(weight_sigma[osl], weight_mu[osl], eps_w[osl], KT, O_S)
        )
        bb.append(
            np.ascontiguousarray(
                np.stack([bias_sigma[osl], bias_mu[osl], eps_b[osl]])[None]
            )
        )

    in_maps = []
    for c in range(N_CORES):
        b, o = divmod(c, O_SHARDS)
        in_maps.append({"xt": xb[b], "wpk": wb[o], "bpk": bb[o]})
    return in_maps


def kernel(x, weight_mu, weight_sigma, bias_mu, bias_sigma, eps_w, eps_b, **run_kwargs):
    from concourse.bass_utils import run_bass_kernel_spmd

    B_S = B_FULL // B_SHARDS
    O_S = OUT_FULL // O_SHARDS
    in_maps = _shard_inputs(
        x, weight_mu, weight_sigma, bias_mu, bias_sigma, eps_w, eps_b
    )
    nc = _get_nc()
    res = run_bass_kernel_spmd(nc, in_maps, core_ids=list(range(N_CORES)), **run_kwargs)
    out = np.empty((B_FULL, OUT_FULL), np.float32)
    for c in range(N_CORES):
        b, o = divmod(c, O_SHARDS)
        out[b * B_S : (b + 1) * B_S, o * O_S : (o + 1) * O_S] = res.results[c]["out"]
    # each (weight, bias) o-shard's KL is identical on both mesh rows; take row 0
    kl_val = np.float32(sum(float(res.results[c]["kl"][0, 0]) for c in range(O_SHARDS)))
    if run_kwargs:
        kernel.last_results = res
    return out, kl_val


# revision 21
# speedup vs baseline: 1.0906x; 1.0494x over previous
"""Bayesian linear layer (reparameterized sample + KL) on 8 Trainium2 NeuronCores.

Reference computation (all fp32):
    weight = weight_mu + exp(weight_sigma) * eps_w          # [OUT, IN]
    bias   = bias_mu   + exp(bias_sigma)   * eps_b          # [OUT]
    out    = x @ weight.T + bias                            # [B, OUT]
    kl     = -0.5 * sum(1 + 2*ws - wm^2 - exp(2*ws))        # over weight
             -0.5 * sum(1 + 2*bs - bm^2 - exp(2*bs))        # over bias

Sharding: 2x4 mesh.  Batch split in 2 (B_S=4096 rows/core), out_features in 4
(O_S=1024 cols/core); core c handles b_shard=c//4, o_shard=c%4.  This gives the
lowest per-core HBM traffic (x shard 67MB + weight params 50MB + out 17MB) and
keeps the kernel PE-bound on float32r matmuls (1 col/cycle at N=512).

Host-side layout work (part of the sharding strategy):
  - x is passed pre-transposed AND pre-tiled as [BT, 128, KT, 128] so each
    batch-tile load is one DMA with 16KB-contiguous per-partition runs.
  - weight mu/sigma/eps are interleaved per k-tile as [KT, 128, 3, O_S]
    (12KB-contiguous per partition) -> one DMA per k-tile.
The contraction dim lands on SBUF partitions with no on-device transposes.

Device kernel per core:
  phase 1: per k-tile, load packed params (scalar-engine HWDGE ring), build
           W^T tile (W = mu + exp(sigma)*eps, rounded once to f32r) and
           accumulate KL partial sums on ACT/DVE.
  phase 2: per 128-row batch tile, load x tile (sync HWDGE ring), accumulate
           out[128, O_S] over 32 k-tiles into PSUM, add bias, store (SWDGE).
  tail:    fold KL partials into one scalar.
"""

import sys

import numpy as np

try:
    import concourse.bass as bass  # noqa: F401
except ImportError:  # pragma: no cover
    sys.path.insert(0, "/opt/trn_rl_repo")

import concourse.bass as bass
import concourse.tile as tile
from concourse import bacc, bass_isa, mybir

P = 128
B_FULL, IN_FULL, OUT_FULL = 8192, 4096, 4096
B_SHARDS, O_SHARDS = 2, 4
N_CORES = 8

F32 = mybir.dt.float32
MM_DT = mybir.dt.float32r  # PE fast-fp32 mode: 1 cycle/row at N>=256

AF = mybir.ActivationFunctionType
OP = mybir.AluOpType
AX = mybir.AxisListType


def build_bayes_kernel(nc, IN, B_S, O_S, mm_dt=MM_DT):
    """Emit the per-core SPMD program. Tensors are declared on nc."""
    KT = IN // P        # k tiles
    BT = B_S // P       # batch tiles
    NFREE = 512         # matmul moving free dim (1 psum bank of fp32)
    OC = O_S // NFREE   # out chunks per batch tile

    xt = nc.dram_tensor("xt", [BT, P, KT, P], mm_dt, kind="ExternalInput").ap()
    wpk = nc.dram_tensor("wpk", [KT, P, 3, O_S], F32, kind="ExternalInput").ap()
    bpk = nc.dram_tensor("bpk", [1, 3, O_S], F32, kind="ExternalInput").ap()
    out = nc.dram_tensor("out", [B_S, O_S], F32, kind="ExternalOutput").ap()
    kl = nc.dram_tensor("kl", [1, 1], F32, kind="ExternalOutput").ap()

    out_r = out.rearrange("(bt p) o -> p bt o", p=P)

    with tile.TileContext(nc) as tc:
        with (
            tc.tile_pool(name="wpool", bufs=1) as wpool,
            tc.tile_pool(name="gen", bufs=2) as gen,
            tc.tile_pool(name="xin", bufs=2) as xin,
            tc.tile_pool(name="ost", bufs=2) as ost,
            tc.tile_pool(name="misc", bufs=1) as misc,
            tc.tile_pool(name="psum", bufs=4, space="PSUM") as psum,
            tc.tile_pool(name="pgps", bufs=1, space="PSUM") as pgps,
        ):
            # Persistent state
            w_tiles = [
                wpool.tile([P, O_S], mm_dt, tag=f"w{kt}", name=f"w{kt}")
                for kt in range(KT)
            ]
            ssig = misc.tile([P, KT], F32, tag="ssig")   # per-ktile sum(sigma)
            smu2 = misc.tile([P, KT], F32, tag="smu2")   # sum(mu^2)
            sv2 = misc.tile([P, KT], F32, tag="sv2")     # sum(exp(2 sigma))
            b_bc = misc.tile([P, O_S], F32, tag="bbc")   # bias broadcast
            klb = misc.tile([1, 4], F32, tag="klb")      # bias kl: ssig, smu2, sv2, tmp

            # Prefetch the first x tiles on the sync ring; they flow while the
            # scalar ring streams the 50MB of weight params, and they feed the
            # PE prologue below.
            PG = min(2, BT)
            xpre = []
            for bt in range(PG):
                xs = xin.tile([P, KT, P], mm_dt, tag="xs", name=f"xpre{bt}")
                nc.sync.dma_start(xs, xt[bt])
                xpre.append(xs)
            pg_ps = [
                [
                    pgps.tile([P, NFREE], F32, tag=f"pg{pb}_{oc}", name=f"pg{pb}_{oc}")
                    for oc in range(OC)
                ]
                for pb in range(PG)
            ]

            # ---- Phase 1: W = mu + exp(sigma)*eps, KL partial sums ----
            # One ACT->DVE hop per k-tile, then all DVE work back-to-back.
            # The first PG batch tiles accumulate their matmuls k-tile by
            # k-tile right here (PE prologue): the PE consumes each W tile as
            # it is produced instead of idling until phase 1 completes.
            for kt in range(KT):
                g = gen.tile([P, 3, O_S], F32, tag="wpk")
                # issue on the sync ring: a DMA instruction's slot-free wait
                # must not sit in front of ACT compute in ACT's in-order queue
                nc.sync.dma_start(g, wpk[kt])
                sig, mu, eps = g[:, 0, :], g[:, 1, :], g[:, 2, :]
                wtmp = gen.tile([P, O_S], F32, tag="wtmp")
                # ACT ops depend only on the DMA (dumps go to dedicated
                # scratch), so ACT's in-order queue never waits on DVE and
                # k-tiles pipeline cleanly.
                d1 = misc.tile([P, O_S], F32, tag="sqd")
                nc.scalar.activation(
                    d1, sig, AF.Identity, accum_out=ssig[:, kt : kt + 1]
                )
                nc.scalar.activation(wtmp, sig, AF.Exp)             # v
                d2 = misc.tile([P, O_S], F32, tag="sqd")
                nc.scalar.activation(
                    d2, mu, AF.Square, accum_out=smu2[:, kt : kt + 1]
                )
                # v^2 summed; tensor output dumps over the dead sigma slot
                nc.vector.scalar_tensor_tensor(
                    sig, wtmp, 1.0, wtmp, OP.mult, OP.mult,
                    accum_out=sv2[:, kt : kt + 1],
                )
                nc.vector.tensor_tensor(wtmp, wtmp, eps, OP.mult)
                # final add writes the f32r matmul operand (single rounding)
                nc.vector.tensor_tensor(w_tiles[kt], wtmp, mu, OP.add)
                for pb in range(PG):
                    for oc in range(OC):
                        sl = slice(oc * NFREE, (oc + 1) * NFREE)
                        nc.tensor.matmul(
                            pg_ps[pb][oc],
                            xpre[pb][:, kt, :],
                            w_tiles[kt][:, sl],
                            start=(kt == 0),
                            stop=(kt == KT - 1),
                        )

            # ---- Bias: value + KL pieces ----
            # all SBUF operands of an op share start partition 0; dead slots
            # of the packed tile double as ACT dump targets.
            bt_ = gen.tile([1, 3, O_S], F32, tag="wpk", name="biastile")
            nc.sync.dma_start(bt_, bpk[0])
            bsig, bmu, beps = bt_[:, 0, :], bt_[:, 1, :], bt_[:, 2, :]
            bv = b_bc[0:1, :]
            nc.vector.tensor_reduce(klb[:, 0:1], bsig, AX.X, OP.add)
            nc.scalar.activation(bv, bsig, AF.Exp)
            nc.vector.tensor_tensor(bv, bv, beps, OP.mult)   # beps dead after
            nc.vector.tensor_tensor(bv, bv, bmu, OP.add)
            nc.scalar.activation(
                beps, bsig, AF.Exp, scale=2.0, accum_out=klb[:, 2:3]
            )
            nc.scalar.activation(bsig, bmu, AF.Square, accum_out=klb[:, 1:2])
            nc.gpsimd.partition_broadcast(b_bc, bv)

            # ---- Prologue eviction: bias-add + store for the PG tiles ----
            for pb in range(PG):
                for oc in range(OC):
                    sl = slice(oc * NFREE, (oc + 1) * NFREE)
                    osb = ost.tile([P, NFREE], F32, tag="osb", name=f"osb_pg{pb}_{oc}")
                    nc.vector.tensor_tensor(osb, pg_ps[pb][oc], b_bc[:, sl], OP.add)
                    nc.sync.dma_start(out_r[:, pb, sl], osb)

            # ---- Phase 2: out[bt] = x[bt] @ W^T + bias ----
            for bt in range(PG, BT):
                xs = xin.tile([P, KT, P], mm_dt, tag="xs")
                nc.sync.dma_start(xs, xt[bt])
                for oc in range(OC):
                    sl = slice(oc * NFREE, (oc + 1) * NFREE)
                    ps = psum.tile([P, NFREE], F32, tag="ps")
                    for kt in range(KT):
                        nc.tensor.matmul(
                            ps,
                            xs[:, kt, :],
                            w_tiles[kt][:, sl],
                            start=(kt == 0),
                            stop=(kt == KT - 1),
                        )
                    osb = ost.tile([P, NFREE], F32, tag="osb")
                    nc.vector.tensor_tensor(osb, ps, b_bc[:, sl], OP.add)
                    nc.sync.dma_start(out_r[:, bt, sl], osb)

            # ---- KL tail ----
            rs = misc.tile([P, 1], F32, tag="rs")
            rm = misc.tile([P, 1], F32, tag="rm")
            rv = misc.tile([P, 1], F32, tag="rv")
            nc.vector.tensor_reduce(rs, ssig, AX.X, OP.add)
            nc.vector.tensor_reduce(rm, smu2, AX.X, OP.add)
            nc.vector.tensor_reduce(rv, sv2, AX.X, OP.add)
            tcol = misc.tile([P, 1], F32, tag="tcol")
            # tcol = 2*rs - rm - rv
            nc.vector.scalar_tensor_tensor(tcol, rs, 2.0, rm, OP.mult, OP.subtract)
            nc.vector.tensor_tensor(tcol, tcol, rv, OP.subtract)
            # bias terms fold into partition 0
            nc.vector.scalar_tensor_tensor(
                klb[:, 3:4], klb[:, 0:1], 2.0, klb[:, 1:2], OP.mult, OP.subtract
            )
            nc.vector.tensor_tensor(klb[:, 3:4], klb[:, 3:4], klb[:, 2:3], OP.subtract)
            nc.vector.tensor_tensor(tcol[0:1, :], tcol[0:1, :], klb[:, 3:4], OP.add)
            tall = misc.tile([P, 1], F32, tag="tall")
            nc.gpsimd.partition_all_reduce(tall, tcol, P, bass_isa.ReduceOp.add)
            # kl = -0.5 * (count + sum(2s - m^2 - v^2))
            count = float(IN * O_S + O_S)
            klt = misc.tile([1, 1], F32, tag="klt")
            nc.vector.tensor_scalar(klt, tall[0:1, :], count, -0.5, OP.add, OP.mult)
            nc.sync.dma_start(kl, klt)


_NC_CACHE = {}


def _get_nc():
    key = "full"
    if key not in _NC_CACHE:
        nc = bacc.Bacc("TRN2", target_bir_lowering=False, debug=False)
        build_bayes_kernel(nc, IN_FULL, B_FULL // B_SHARDS, OUT_FULL // O_SHARDS)
        nc.compile()
        _NC_CACHE[key] = nc
    return _NC_CACHE[key]


def _pack_x(x_shard, BT, KT):
    # [B_S, IN] -> [BT, P, KT, P] with [bt, p(=feature in tile), kt, b]
    x4 = x_shard.reshape(BT, P, KT, P)          # [bt, b, kt, p_feature]
    return np.ascontiguousarray(x4.transpose(0, 3, 2, 1))


def _pack_w(sig, mu, eps, KT, O_S):
    # each [O_S, IN] -> packed [KT, P, 3, O_S] with feature on partition
    stk = np.stack([sig.T, mu.T, eps.T], axis=1)   # [IN, 3, O_S]
    return np.ascontiguousarray(stk.reshape(KT, P, 3, O_S))


def _shard_inputs(x, weight_mu, weight_sigma, bias_mu, bias_sigma, eps_w, eps_b):
    B_S = B_FULL // B_SHARDS
    O_S = OUT_FULL // O_SHARDS
    BT, KT = B_S // P, IN_FULL // P
    f = np.float32
    x = np.asarray(x, dtype=f)
    weight_mu = np.asarray(weight_mu, dtype=f)
    weight_sigma = np.asarray(weight_sigma, dtype=f)
    eps_w = np.asarray(eps_w, dtype=f)
    bias_mu = np.asarray(bias_mu, dtype=f)
    bias_sigma = np.asarray(bias_sigma, dtype=f)
    eps_b = np.asarray(eps_b, dtype=f)

    xb = [_pack_x(x[b * B_S : (b + 1) * B_S], BT, KT) for b in range(B_SHARDS)]
    wb, bb = [], []
    for o in range(O_SHARDS):
        osl = slice(o * O_S, (o + 1) * O_S)
        wb.append(
            _pack_w(weight_sigma[osl], weight_mu[osl], eps_w[osl], KT, O_S)
        )
        bb.append(
            np.ascontiguousarray(
                np.stack([bias_sigma[osl], bias_mu[osl], eps_b[osl]])[None]
            )
        )

    in_maps = []
    for c in range(N_CORES):
        b, o = divmod(c, O_SHARDS)
        in_maps.append({"xt": xb[b], "wpk": wb[o], "bpk": bb[o]})
    return in_maps


def kernel(x, weight_mu, weight_sigma, bias_mu, bias_sigma, eps_w, eps_b, **run_kwargs):
    from concourse.bass_utils import run_bass_kernel_spmd

    B_S = B_FULL // B_SHARDS
    O_S = OUT_FULL // O_SHARDS
    in_maps = _shard_inputs(
        x, weight_mu, weight_sigma, bias_mu, bias_sigma, eps_w, eps_b
    )
    nc = _get_nc()
    res = run_bass_kernel_spmd(nc, in_maps, core_ids=list(range(N_CORES)), **run_kwargs)
    out = np.empty((B_FULL, OUT_FULL), np.float32)
    for c in range(N_CORES):
        b, o = divmod(c, O_SHARDS)
        out[b * B_S : (b + 1) * B_S, o * O_S : (o + 1) * O_S] = res.results[c]["out"]
    # each (weight, bias) o-shard's KL is identical on both mesh rows; take row 0
    kl_val = np.float32(sum(float(res.results[c]["kl"][0, 0]) for c in range(O_SHARDS)))
    if run_kwargs:
        kernel.last_results = res
    return out, kl_val


# revision 22
# speedup vs baseline: 1.3031x; 1.1949x over previous
"""Bayesian linear layer (reparameterized sample + KL) on 8 Trainium2 NeuronCores.

Reference computation (all fp32):
    weight = weight_mu + exp(weight_sigma) * eps_w          # [OUT, IN]
    bias   = bias_mu   + exp(bias_sigma)   * eps_b          # [OUT]
    out    = x @ weight.T + bias                            # [B, OUT]
    kl     = -0.5 * sum(1 + 2*ws - wm^2 - exp(2*ws))        # over weight
             -0.5 * sum(1 + 2*bs - bm^2 - exp(2*bs))        # over bias

Sharding: column-parallel 1x8.  out_features split in 8 (O_S=512 per core), x
replicated.  Per core: W^T shard [4096, 512] stays resident in SBUF (64KB per
partition, f32r), x streams through in 64 batch tiles.  The weight-param
stream is only 25MB/core so the W-generation phase is short, and 4 batch
tiles run a PE "prologue" during it (k-sliced PSUM accumulation across all 8
banks) so the tensor engine never goes cold.

Host-side layout work (part of the sharding strategy):
  - x is passed pre-transposed AND pre-tiled as [64, 128, 32, 128]
    ([batch-tile, feature-in-ktile, ktile, batch-in-tile]) so each batch-tile
    load is one DMA with 16KB-contiguous per-partition runs.
  - weight mu/sigma/eps shards are interleaved per k-tile as [32, 128, 3, 512]
    (6KB-contiguous per partition) -> one DMA per k-tile.
The contraction dim lands on SBUF partitions with no on-device transposes.

Matmuls run in float32r (the PE's full-rate fp32 mode, tf32-like operand
rounding, fp32 PSUM accumulation).
"""

import sys

import numpy as np

try:
    import concourse.bass as bass  # noqa: F401
except ImportError:  # pragma: no cover
    sys.path.insert(0, "/opt/trn_rl_repo")

import concourse.bass as bass
import concourse.tile as tile
from concourse import bacc, bass_isa, mybir

P = 128
B_FULL, IN_FULL, OUT_FULL = 8192, 4096, 4096
O_SHARDS = 8
N_CORES = 8

F32 = mybir.dt.float32
MM_DT = mybir.dt.float32r  # PE fast-fp32 mode: 1 cycle/row at N>=256

AF = mybir.ActivationFunctionType
OP = mybir.AluOpType
AX = mybir.AxisListType


def build_bayes_kernel(nc, IN, B_S, O_S, mm_dt=MM_DT):
    """Emit the per-core SPMD program. Tensors are declared on nc."""
    KT = IN // P        # k tiles
    BT = B_S // P       # batch tiles
    assert O_S <= 512   # one psum bank per out tile

    xt = nc.dram_tensor("xt", [BT, P, KT, P], mm_dt, kind="ExternalInput").ap()
    wpk = nc.dram_tensor("wpk", [KT, P, 3, O_S], F32, kind="ExternalInput").ap()
    bpk = nc.dram_tensor("bpk", [1, 3, O_S], F32, kind="ExternalInput").ap()
    out = nc.dram_tensor("out", [B_S, O_S], F32, kind="ExternalOutput").ap()
    kl = nc.dram_tensor("kl", [1, 1], F32, kind="ExternalOutput").ap()

    out_r = out.rearrange("(bt p) o -> p bt o", p=P)

    with tile.TileContext(nc) as tc:
        with (
            tc.tile_pool(name="wpool", bufs=1) as wpool,
            tc.tile_pool(name="gen", bufs=4) as gen,
            tc.tile_pool(name="xin", bufs=5) as xin,
            tc.tile_pool(name="ost", bufs=3) as ost,
            tc.tile_pool(name="misc", bufs=1) as misc,
            tc.tile_pool(name="psum", bufs=4, space="PSUM") as psum,
            tc.tile_pool(name="pgps", bufs=1, space="PSUM") as pgps,
        ):
            # Persistent state
            w_tiles = [
                wpool.tile([P, O_S], mm_dt, tag=f"w{kt}", name=f"w{kt}")
                for kt in range(KT)
            ]
            ssig = misc.tile([P, KT], F32, tag="ssig")   # per-ktile sum(sigma)
            smu2 = misc.tile([P, KT], F32, tag="smu2")   # sum(mu^2)
            sv2 = misc.tile([P, KT], F32, tag="sv2")     # sum(exp(2 sigma))
            b_bc = misc.tile([P, O_S], F32, tag="bbc")   # bias broadcast
            klb = misc.tile([1, 4], F32, tag="klb")      # bias kl: ssig, smu2, sv2, tmp

            # Prefetch x tiles on the sync ring; they feed the PE prologue
            # that runs inside phase 1.
            PG = min(4, BT)
            xpre = []
            for bt in range(PG):
                xs = xin.tile([P, KT, P], mm_dt, tag="xs", name=f"xpre{bt}")
                nc.sync.dma_start(xs, xt[bt])
                xpre.append(xs)
            pg_ps = [
                pgps.tile([P, O_S], F32, tag=f"pg{pb}", name=f"pg{pb}")
                for pb in range(PG)
            ]

            # ---- Phase 1: W = mu + exp(sigma)*eps, KL partial sums ----
            # ACT ops depend only on the k-tile's DMA (dumps go to dedicated
            # scratch), DVE work runs back-to-back, and the 4-deep gen pool
            # pipelines the DMA->ACT->DVE chain across k-tiles.  The first PG
            # batch tiles accumulate their matmuls k-tile by k-tile here so
            # the PE consumes each W tile as it is produced.
            for kt in range(KT):
                g = gen.tile([P, 3, O_S], F32, tag="wpk")
                nc.sync.dma_start(g, wpk[kt])
                sig, mu, eps = g[:, 0, :], g[:, 1, :], g[:, 2, :]
                wtmp = gen.tile([P, O_S], F32, tag="wtmp")
                nc.scalar.activation(wtmp, sig, AF.Exp)             # v
                d2 = misc.tile([P, O_S], F32, tag="sqd")
                nc.scalar.activation(
                    d2, mu, AF.Square, accum_out=smu2[:, kt : kt + 1]
                )
                nc.vector.tensor_reduce(ssig[:, kt : kt + 1], sig, AX.X, OP.add)
                # v^2 summed; tensor output dumps over the dead sigma slot
                nc.vector.scalar_tensor_tensor(
                    sig, wtmp, 1.0, wtmp, OP.mult, OP.mult,
                    accum_out=sv2[:, kt : kt + 1],
                )
                nc.vector.tensor_tensor(wtmp, wtmp, eps, OP.mult)
                # final add writes the f32r matmul operand (single rounding)
                nc.vector.tensor_tensor(w_tiles[kt], wtmp, mu, OP.add)
                for pb in range(PG):
                    nc.tensor.matmul(
                        pg_ps[pb],
                        xpre[pb][:, kt, :],
                        w_tiles[kt],
                        start=(kt == 0),
                        stop=(kt == KT - 1),
                    )

            # ---- Bias: value + KL pieces ----
            # all SBUF operands of an op share start partition 0; dead slots
            # of the packed tile double as ACT dump targets.
            bt_ = gen.tile([1, 3, O_S], F32, tag="wpk", name="biastile")
            nc.sync.dma_start(bt_, bpk[0])
            bsig, bmu, beps = bt_[:, 0, :], bt_[:, 1, :], bt_[:, 2, :]
            bv = b_bc[0:1, :]
            nc.vector.tensor_reduce(klb[:, 0:1], bsig, AX.X, OP.add)
            nc.scalar.activation(bv, bsig, AF.Exp)
            nc.vector.tensor_tensor(bv, bv, beps, OP.mult)   # beps dead after
            nc.vector.tensor_tensor(bv, bv, bmu, OP.add)
            nc.scalar.activation(
                beps, bsig, AF.Exp, scale=2.0, accum_out=klb[:, 2:3]
            )
            nc.scalar.activation(bsig, bmu, AF.Square, accum_out=klb[:, 1:2])
            nc.gpsimd.partition_broadcast(b_bc, bv)

            # ---- Prologue eviction: bias-add + store for the PG tiles ----
            for pb in range(PG):
                osb = ost.tile([P, O_S], F32, tag="osb", name=f"osb_pg{pb}")
                nc.vector.tensor_tensor(osb, pg_ps[pb], b_bc, OP.add)
                nc.sync.dma_start(out_r[:, pb, :], osb)

            # ---- Phase 2: out[bt] = x[bt] @ W^T + bias ----
            for bt in range(PG, BT):
                xs = xin.tile([P, KT, P], mm_dt, tag="xs")
                nc.sync.dma_start(xs, xt[bt])
                ps = psum.tile([P, O_S], F32, tag="ps")
                for kt in range(KT):
                    nc.tensor.matmul(
                        ps,
                        xs[:, kt, :],
                        w_tiles[kt],
                        start=(kt == 0),
                        stop=(kt == KT - 1),
                    )
                osb = ost.tile([P, O_S], F32, tag="osb")
                nc.vector.tensor_tensor(osb, ps, b_bc, OP.add)
                nc.sync.dma_start(out_r[:, bt, :], osb)

            # ---- KL tail ----
            rs = misc.tile([P, 1], F32, tag="rs")
            rm = misc.tile([P, 1], F32, tag="rm")
            rv = misc.tile([P, 1], F32, tag="rv")
            nc.vector.tensor_reduce(rs, ssig, AX.X, OP.add)
            nc.vector.tensor_reduce(rm, smu2, AX.X, OP.add)
            nc.vector.tensor_reduce(rv, sv2, AX.X, OP.add)
            tcol = misc.tile([P, 1], F32, tag="tcol")
            # tcol = 2*rs - rm - rv
            nc.vector.scalar_tensor_tensor(tcol, rs, 2.0, rm, OP.mult, OP.subtract)
            nc.vector.tensor_tensor(tcol, tcol, rv, OP.subtract)
            # bias terms fold into partition 0
            nc.vector.scalar_tensor_tensor(
                klb[:, 3:4], klb[:, 0:1], 2.0, klb[:, 1:2], OP.mult, OP.subtract
            )
            nc.vector.tensor_tensor(klb[:, 3:4], klb[:, 3:4], klb[:, 2:3], OP.subtract)
            nc.vector.tensor_tensor(tcol[0:1, :], tcol[0:1, :], klb[:, 3:4], OP.add)
            tall = misc.tile([P, 1], F32, tag="tall")
            nc.gpsimd.partition_all_reduce(tall, tcol, P, bass_isa.ReduceOp.add)
            # kl = -0.5 * (count + sum(2s - m^2 - v^2))
            count = float(IN * O_S + O_S)
            klt = misc.tile([1, 1], F32, tag="klt")
            nc.vector.tensor_scalar(klt, tall[0:1, :], count, -0.5, OP.add, OP.mult)
            nc.sync.dma_start(kl, klt)


_NC_CACHE = {}


def _get_nc():
    key = "full"
    if key not in _NC_CACHE:
        nc = bacc.Bacc("TRN2", target_bir_lowering=False, debug=False)
        build_bayes_kernel(nc, IN_FULL, B_FULL, OUT_FULL // O_SHARDS)
        nc.compile()
        _NC_CACHE[key] = nc
    return _NC_CACHE[key]


def _pack_x(x_full, BT, KT):
    # [B, IN] -> [BT, P, KT, P] with [bt, p(=feature in tile), kt, b]
    x4 = x_full.reshape(BT, P, KT, P)           # [bt, b, kt, p_feature]
    return np.ascontiguousarray(x4.transpose(0, 3, 2, 1))


def _pack_w(sig, mu, eps, KT, O_S):
    # each [O_S, IN] -> packed [KT, P, 3, O_S] with feature on partition
    stk = np.stack([sig.T, mu.T, eps.T], axis=1)   # [IN, 3, O_S]
    return np.ascontiguousarray(stk.reshape(KT, P, 3, O_S))


def _shard_inputs(x, weight_mu, weight_sigma, bias_mu, bias_sigma, eps_w, eps_b):
    O_S = OUT_FULL // O_SHARDS
    BT, KT = B_FULL // P, IN_FULL // P
    f = np.float32
    x = np.asarray(x, dtype=f)
    weight_mu = np.asarray(weight_mu, dtype=f)
    weight_sigma = np.asarray(weight_sigma, dtype=f)
    eps_w = np.asarray(eps_w, dtype=f)
    bias_mu = np.asarray(bias_mu, dtype=f)
    bias_sigma = np.asarray(bias_sigma, dtype=f)
    eps_b = np.asarray(eps_b, dtype=f)

    xb = _pack_x(x, BT, KT)
    in_maps = []
    for o in range(N_CORES):
        osl = slice(o * O_S, (o + 1) * O_S)
        in_maps.append(
            {
                "xt": xb,
                "wpk": _pack_w(weight_sigma[osl], weight_mu[osl], eps_w[osl], KT, O_S),
                "bpk": np.ascontiguousarray(
                    np.stack([bias_sigma[osl], bias_mu[osl], eps_b[osl]])[None]
                ),
            }
        )
    return in_maps


def kernel(x, weight_mu, weight_sigma, bias_mu, bias_sigma, eps_w, eps_b, **run_kwargs):
    from concourse.bass_utils import run_bass_kernel_spmd

    O_S = OUT_FULL // O_SHARDS
    in_maps = _shard_inputs(
        x, weight_mu, weight_sigma, bias_mu, bias_sigma, eps_w, eps_b
    )
    nc = _get_nc()
    res = run_bass_kernel_spmd(nc, in_maps, core_ids=list(range(N_CORES)), **run_kwargs)
    out = np.empty((B_FULL, OUT_FULL), np.float32)
    kl_val = 0.0
    for c in range(N_CORES):
        out[:, c * O_S : (c + 1) * O_S] = res.results[c]["out"]
        kl_val += float(res.results[c]["kl"][0, 0])
    if run_kwargs:
        kernel.last_results = res
    return out, np.float32(kl_val)


# revision 25
# speedup vs baseline: 1.4105x; 1.0825x over previous
"""Bayesian linear layer (reparameterized sample + KL) on 8 Trainium2 NeuronCores.

Reference computation (all fp32):
    weight = weight_mu + exp(weight_sigma) * eps_w          # [OUT, IN]
    bias   = bias_mu   + exp(bias_sigma)   * eps_b          # [OUT]
    out    = x @ weight.T + bias                            # [B, OUT]
    kl     = -0.5 * sum(1 + 2*ws - wm^2 - exp(2*ws))        # over weight
             -0.5 * sum(1 + 2*bs - bm^2 - exp(2*bs))        # over bias

Sharding: column-parallel 1x8.  out_features split in 8 (O_S=512 per core), x
replicated.  Per core: W^T shard [4096, 512] stays resident in SBUF (64KB per
partition, f32r), x streams through in 64 batch tiles.  The weight-param
stream is only 25MB/core so the W-generation phase is short, and 4 batch
tiles run a PE "prologue" during it (k-sliced PSUM accumulation across all 8
banks) so the tensor engine never goes cold.

Host-side layout work (part of the sharding strategy):
  - x is passed pre-transposed AND pre-tiled as [64, 128, 32, 128]
    ([batch-tile, feature-in-ktile, ktile, batch-in-tile]) so each batch-tile
    load is one DMA with 16KB-contiguous per-partition runs.
  - weight mu/sigma/eps shards are interleaved per k-tile as [32, 128, 3, 512]
    (6KB-contiguous per partition) -> one DMA per k-tile.
The contraction dim lands on SBUF partitions with no on-device transposes.

Matmuls run in float32r (the PE's full-rate fp32 mode, tf32-like operand
rounding, fp32 PSUM accumulation).
"""

import sys

import numpy as np

try:
    import concourse.bass as bass  # noqa: F401
except ImportError:  # pragma: no cover
    sys.path.insert(0, "/opt/trn_rl_repo")

import concourse.bass as bass
import concourse.tile as tile
from concourse import bacc, bass_isa, mybir

P = 128
B_FULL, IN_FULL, OUT_FULL = 8192, 4096, 4096
O_SHARDS = 8
N_CORES = 8

F32 = mybir.dt.float32
# fp16 operands: 11-bit significand, the same precision class as the PE's
# tf32-like f32r operand rounding, at half the DMA bytes and SBUF footprint.
X_DT = mybir.dt.float16
W_DT = mybir.dt.float16
MM_DT = X_DT

AF = mybir.ActivationFunctionType
OP = mybir.AluOpType
AX = mybir.AxisListType


def build_bayes_kernel(nc, IN, B_S, O_S, mm_dt=X_DT, w_dt=W_DT):
    """Emit the per-core SPMD program. Tensors are declared on nc."""
    KT = IN // P        # k tiles
    BT = B_S // P       # batch tiles
    assert O_S <= 512   # one psum bank per out tile

    xt = nc.dram_tensor("xt", [BT, P, KT, P], mm_dt, kind="ExternalInput").ap()
    wpk = nc.dram_tensor("wpk", [KT, P, 3, O_S], F32, kind="ExternalInput").ap()
    bpk = nc.dram_tensor("bpk", [1, 3, O_S], F32, kind="ExternalInput").ap()
    out = nc.dram_tensor("out", [B_S, O_S], F32, kind="ExternalOutput").ap()
    kl = nc.dram_tensor("kl", [1, 1], F32, kind="ExternalOutput").ap()

    out_r = out.rearrange("(bt p) o -> p bt o", p=P)

    with tile.TileContext(nc) as tc:
        with (
            tc.tile_pool(name="wpool", bufs=1) as wpool,
            tc.tile_pool(name="gen", bufs=4) as gen,
            tc.tile_pool(name="xin", bufs=5) as xin,
            tc.tile_pool(name="ost", bufs=3) as ost,
            tc.tile_pool(name="misc", bufs=1) as misc,
            tc.tile_pool(name="psum", bufs=4, space="PSUM") as psum,
            tc.tile_pool(name="pgps", bufs=1, space="PSUM") as pgps,
        ):
            # Persistent state
            w_tiles = [
                wpool.tile([P, O_S], w_dt, tag=f"w{kt}", name=f"w{kt}")
                for kt in range(KT)
            ]
            ssig = misc.tile([P, KT], F32, tag="ssig")   # per-ktile sum(sigma)
            smu2 = misc.tile([P, KT], F32, tag="smu2")   # sum(mu^2)
            sv2 = misc.tile([P, KT], F32, tag="sv2")     # sum(exp(2 sigma))
            b_bc = misc.tile([P, O_S], F32, tag="bbc")   # bias broadcast
            klb = misc.tile([1, 4], F32, tag="klb")      # bias kl: ssig, smu2, sv2, tmp

            # Prefetch x tiles on the sync ring; they feed the PE prologue
            # that runs inside phase 1.
            PG = min(4, BT)
            xpre = []
            for bt in range(PG):
                xs = xin.tile([P, KT, P], mm_dt, tag="xs", name=f"xpre{bt}")
                nc.sync.dma_start(xs, xt[bt])
                xpre.append(xs)
            pg_ps = [
                pgps.tile([P, O_S], F32, tag=f"pg{pb}", name=f"pg{pb}")
                for pb in range(PG)
            ]

            # ---- Phase 1: W = mu + exp(sigma)*eps, KL partial sums ----
            # ACT ops depend only on the k-tile's DMA (dumps go to dedicated
            # scratch), DVE work runs back-to-back, and the 4-deep gen pool
            # pipelines the DMA->ACT->DVE chain across k-tiles.  The first PG
            # batch tiles accumulate their matmuls k-tile by k-tile here so
            # the PE consumes each W tile as it is produced.
            for kt in range(KT):
                g = gen.tile([P, 3, O_S], F32, tag="wpk")
                nc.sync.dma_start(g, wpk[kt])
                sig, mu, eps = g[:, 0, :], g[:, 1, :], g[:, 2, :]
                wtmp = gen.tile([P, O_S], F32, tag="wtmp")
                nc.scalar.activation(wtmp, sig, AF.Exp)             # v
                d2 = misc.tile([P, O_S], F32, tag="sqd")
                nc.scalar.activation(
                    d2, mu, AF.Square, accum_out=smu2[:, kt : kt + 1]
                )
                nc.vector.tensor_reduce(ssig[:, kt : kt + 1], sig, AX.X, OP.add)
                # v^2 summed; tensor output dumps over the dead sigma slot
                nc.vector.scalar_tensor_tensor(
                    sig, wtmp, 1.0, wtmp, OP.mult, OP.mult,
                    accum_out=sv2[:, kt : kt + 1],
                )
                nc.vector.tensor_tensor(wtmp, wtmp, eps, OP.mult)
                # final add writes the f32r matmul operand (single rounding)
                nc.vector.tensor_tensor(w_tiles[kt], wtmp, mu, OP.add)
                for pb in range(PG):
                    nc.tensor.matmul(
                        pg_ps[pb],
                        xpre[pb][:, kt, :],
                        w_tiles[kt],
                        start=(kt == 0),
                        stop=(kt == KT - 1),
                    )

            # ---- Bias: value + KL pieces ----
            # all SBUF operands of an op share start partition 0; dead slots
            # of the packed tile double as ACT dump targets.
            bt_ = gen.tile([1, 3, O_S], F32, tag="wpk", name="biastile")
            nc.sync.dma_start(bt_, bpk[0])
            bsig, bmu, beps = bt_[:, 0, :], bt_[:, 1, :], bt_[:, 2, :]
            bv = b_bc[0:1, :]
            nc.vector.tensor_reduce(klb[:, 0:1], bsig, AX.X, OP.add)
            nc.scalar.activation(bv, bsig, AF.Exp)
            nc.vector.tensor_tensor(bv, bv, beps, OP.mult)   # beps dead after
            nc.vector.tensor_tensor(bv, bv, bmu, OP.add)
            nc.scalar.activation(
                beps, bsig, AF.Exp, scale=2.0, accum_out=klb[:, 2:3]
            )
            nc.scalar.activation(bsig, bmu, AF.Square, accum_out=klb[:, 1:2])
            nc.gpsimd.partition_broadcast(b_bc, bv)

            # ---- Prologue eviction: bias-add + store for the PG tiles ----
            for pb in range(PG):
                osb = ost.tile([P, O_S], F32, tag="osb", name=f"osb_pg{pb}")
                nc.vector.tensor_tensor(osb, pg_ps[pb], b_bc, OP.add)
                nc.sync.dma_start(out_r[:, pb, :], osb)

            # ---- Phase 2: out[bt] = x[bt] @ W^T + bias ----
            for bt in range(PG, BT):
                xs = xin.tile([P, KT, P], mm_dt, tag="xs")
                nc.sync.dma_start(xs, xt[bt])
                ps = psum.tile([P, O_S], F32, tag="ps")
                for kt in range(KT):
                    nc.tensor.matmul(
                        ps,
                        xs[:, kt, :],
                        w_tiles[kt],
                        start=(kt == 0),
                        stop=(kt == KT - 1),
                    )
                osb = ost.tile([P, O_S], F32, tag="osb")
                nc.vector.tensor_tensor(osb, ps, b_bc, OP.add)
                nc.sync.dma_start(out_r[:, bt, :], osb)

            # ---- KL tail ----
            rs = misc.tile([P, 1], F32, tag="rs")
            rm = misc.tile([P, 1], F32, tag="rm")
            rv = misc.tile([P, 1], F32, tag="rv")
            nc.vector.tensor_reduce(rs, ssig, AX.X, OP.add)
            nc.vector.tensor_reduce(rm, smu2, AX.X, OP.add)
            nc.vector.tensor_reduce(rv, sv2, AX.X, OP.add)
            tcol = misc.tile([P, 1], F32, tag="tcol")
            # tcol = 2*rs - rm - rv
            nc.vector.scalar_tensor_tensor(tcol, rs, 2.0, rm, OP.mult, OP.subtract)
            nc.vector.tensor_tensor(tcol, tcol, rv, OP.subtract)
            # bias terms fold into partition 0
            nc.vector.scalar_tensor_tensor(
                klb[:, 3:4], klb[:, 0:1], 2.0, klb[:, 1:2], OP.mult, OP.subtract
            )
            nc.vector.tensor_tensor(klb[:, 3:4], klb[:, 3:4], klb[:, 2:3], OP.subtract)
            nc.vector.tensor_tensor(tcol[0:1, :], tcol[0:1, :], klb[:, 3:4], OP.add)
            tall = misc.tile([P, 1], F32, tag="tall")
            nc.gpsimd.partition_all_reduce(tall, tcol, P, bass_isa.ReduceOp.add)
            # kl = -0.5 * (count + sum(2s - m^2 - v^2))
            count = float(IN * O_S + O_S)
            klt = misc.tile([1, 1], F32, tag="klt")
            nc.vector.tensor_scalar(klt, tall[0:1, :], count, -0.5, OP.add, OP.mult)
            nc.sync.dma_start(kl, klt)


_NC_CACHE = {}


def _get_nc():
    key = "full"
    if key not in _NC_CACHE:
        nc = bacc.Bacc("TRN2", target_bir_lowering=False, debug=False)
        build_bayes_kernel(nc, IN_FULL, B_FULL, OUT_FULL // O_SHARDS)
        nc.compile()
        _NC_CACHE[key] = nc
    return _NC_CACHE[key]


def _pack_x(x_full, BT, KT):
    # [B, IN] -> [BT, P, KT, P] with [bt, p(=feature in tile), kt, b], fp16
    x4 = x_full.reshape(BT, P, KT, P)           # [bt, b, kt, p_feature]
    return np.ascontiguousarray(x4.transpose(0, 3, 2, 1).astype(np.float16))


def _pack_w(sig, mu, eps, KT, O_S):
    # each [O_S, IN] -> packed [KT, P, 3, O_S] with feature on partition
    stk = np.stack([sig.T, mu.T, eps.T], axis=1)   # [IN, 3, O_S]
    return np.ascontiguousarray(stk.reshape(KT, P, 3, O_S))


def _shard_inputs(x, weight_mu, weight_sigma, bias_mu, bias_sigma, eps_w, eps_b):
    O_S = OUT_FULL // O_SHARDS
    BT, KT = B_FULL // P, IN_FULL // P
    f = np.float32
    x = np.asarray(x, dtype=f)
    weight_mu = np.asarray(weight_mu, dtype=f)
    weight_sigma = np.asarray(weight_sigma, dtype=f)
    eps_w = np.asarray(eps_w, dtype=f)
    bias_mu = np.asarray(bias_mu, dtype=f)
    bias_sigma = np.asarray(bias_sigma, dtype=f)
    eps_b = np.asarray(eps_b, dtype=f)

    xb = _pack_x(x, BT, KT)
    in_maps = []
    for o in range(N_CORES):
        osl = slice(o * O_S, (o + 1) * O_S)
        in_maps.append(
            {
                "xt": xb,
                "wpk": _pack_w(weight_sigma[osl], weight_mu[osl], eps_w[osl], KT, O_S),
                "bpk": np.ascontiguousarray(
                    np.stack([bias_sigma[osl], bias_mu[osl], eps_b[osl]])[None]
                ),
            }
        )
    return in_maps


def kernel(x, weight_mu, weight_sigma, bias_mu, bias_sigma, eps_w, eps_b, **run_kwargs):
    from concourse.bass_utils import run_bass_kernel_spmd

    O_S = OUT_FULL // O_SHARDS
    in_maps = _shard_inputs(
        x, weight_mu, weight_sigma, bias_mu, bias_sigma, eps_w, eps_b
    )
    nc = _get_nc()
    res = run_bass_kernel_spmd(nc, in_maps, core_ids=list(range(N_CORES)), **run_kwargs)
    out = np.empty((B_FULL, OUT_FULL), np.float32)
    kl_val = 0.0
    for c in range(N_CORES):
        out[:, c * O_S : (c + 1) * O_S] = res.results[c]["out"]
        kl_val += float(res.results[c]["kl"][0, 0])
    if run_kwargs:
        kernel.last_results = res
    return out, np.float32(kl_val)
